# revision 61
# baseline (speedup 1.0000x reference)
"""GCNConv (fc+ReLU -> GCN propagate) distributed over 8 trn2 NeuronCores.

Strategy (graph/data parallel, per the sharding hint):
- Nodes sharded contiguously: core c owns nodes [c*12500, (c+1)*12500).
- Each core computes its shard of g = (relu(X@W1+b1) @ W2) * dinv on the
  tensor engine, then the g table (bf16) is AllGathered to every core's HBM.
- Edges are bucketed by dst shard (host side). Per core the edges are
  grouped by (dst tile of 128, src-table chunk, src parity) with
  program-uniform padded block counts so one SPMD program serves all cores.
- Messages are fetched with dma_gather (SDMA MoE gather) from the pair-packed
  table [50000, 128]bf16 (two nodes per 256B row, int16 chunk-relative
  indices), landing edge-major [128, nblk, 128]. The AllGather is split in
  two half-table collectives so chunk-0 gathers start earlier. Buckets are
  (dst tile, chunk) with a per-edge DVE parity select (lo + (hi-lo)*mask)
  picking the node half of each gathered pair row. Each bucket-group call is
  split into SPLITS sub-calls rotated over the 4 SWDGE queues (desc-gen for
  different queues overlaps on different Q7 core pairs; DMA completion is
  tracked with one semaphore per queue since cross-queue completions are
  unordered) and NSETS gbuf/S/gsel buffer sets pipeline desc-gen against
  drain + DVE + PE.
- Scatter-add runs on the tensor engine: per 128-edge block a one-hot
  S[edge, dstslot] (built on DVE via broadcast is_equal) multiplies the
  parity-selected block into the dst tile's PSUM accumulator. Self-loops are
  a per-tile identity matmul of the core's own g tile.
- Epilogue: ACT scales by dinv[dst] per partition, DVE adds b2, DMA out.
"""
import os
import numpy as np
import ml_dtypes

import concourse.bass as bass
import concourse.bacc as bacc
import concourse.mybir as mybir
from concourse.library_config import mlp as mlp_library
from concourse.bass_utils import run_bass_kernel_spmd

NCORES = 8
N = 100000
E_TOTAL = 1600000
IN_C = 64
HID = 128
OUT_C = 64
NSH = N // NCORES            # 12500 nodes per core
P = 128
NT = 98                      # dst tiles per core (97 full + 84)
NPAD = NT * P                # 12544 padded nodes per core
LAST_ROWS = NSH - 97 * P     # 84
PAIRS = N // 2               # 50000 pair rows
CHUNK = PAIRS // 2           # 25000 rows per gather chunk (int16 range)
TG = 7                       # tiles per group
NGRP = NT // TG              # 14 groups
NCALL = NGRP * 2             # 28 gather calls (group x chunk)

FP32 = mybir.dt.float32
BF16 = mybir.dt.bfloat16
I16 = mybir.dt.int16

LAST_EXEC_NS = [None]
LAST_TRACE = [None]


def _host_prep(node_features, edge_index, W1, b1, W2, b2):
    X = np.asarray(node_features, dtype=np.float32)
    ei = np.asarray(edge_index)
    W1 = np.asarray(W1, dtype=np.float32)
    b1 = np.asarray(b1, dtype=np.float32)
    W2 = np.asarray(W2, dtype=np.float32)
    b2 = np.asarray(b2, dtype=np.float32)
    src = ei[0].astype(np.int64)
    dst = ei[1].astype(np.int64)
    E = src.shape[0]

    core = dst // NSH
    dloc = dst - core * NSH
    t = dloc >> 7
    slot = dloc & 127
    # src table row mapping for the split AllGather: chunk h = which half of
    # the owning core's shard; row within chunk = src_core*3125 + local pair.
    scor = src // NSH
    sloc = src - scor * NSH
    half = (sloc >= (NSH // 2)).astype(np.int64)
    j = sloc - half * (NSH // 2)
    par = src & 1
    chunk = half
    stream = chunk                                # chunk only; parity via mask
    pairrel = (scor * (NSH // 4) + (j >> 1)).astype(np.int16)

    # per (core, t, stream) counts -> uniform block counts
    kf = ((core * NT + t) * 2 + stream)
    counts = np.bincount(kf, minlength=NCORES * NT * 2).reshape(NCORES, NT, 2)
    NB = np.maximum(np.ceil(counts.max(axis=0) / P).astype(np.int64), 0)  # [NT, 2]

    # call layout: call k = 2*g + c covers tiles 7g..7g+6, stream c
    NBC = np.zeros(NCALL, dtype=np.int64)
    seg_block_base = np.zeros((NT, 2), dtype=np.int64)  # block offset within call
    for g in range(NGRP):
        for c in range(2):
            k = 2 * g + c
            off = 0
            for tt in range(g * TG, (g + 1) * TG):
                seg_block_base[tt, c] = off
                off += NB[tt, c]
            NBC[k] = off
    NBTOT = int(NBC.sum())
    call_block_base = np.concatenate([[0], np.cumsum(NBC)])[:-1]   # blocks before call k

    # global slot position of each (t, s) segment
    k_of = np.zeros((NT, 2), dtype=np.int64)
    for tt in range(NT):
        for s in range(2):
            k_of[tt, s] = 2 * (tt // TG) + s
    GPOS = (call_block_base[k_of] + seg_block_base) * P            # [NT, 2]

    # rank of each edge within its (core, t, s) bucket
    order = np.argsort(kf, kind="stable")
    cnt_flat = np.bincount(kf, minlength=NCORES * NT * 2)
    start_flat = np.concatenate([[0], np.cumsum(cnt_flat)])[:-1]
    rank = np.empty(E, dtype=np.int64)
    rank[order] = np.arange(E) - start_flat[kf[order]]

    gslot = GPOS[t, stream] + rank                                  # [E]
    Bg = gslot >> 7
    prow = gslot & 127

    # dstslot array [cores, 128, NBTOT] bf16 (pad = -1)
    dslot_np = np.full((NCORES, P, NBTOT), -1.0, dtype=np.float32)
    dslot_np[core, prow, Bg] = slot
    dslot_np = dslot_np.astype(ml_dtypes.bfloat16)

    # parity mask [cores, 128, NBTOT] bf16 (1 -> use odd half of pair row)
    pmask_np = np.zeros((NCORES, P, NBTOT), dtype=np.float32)
    pmask_np[core, prow, Bg] = par
    pmask_np = pmask_np.astype(ml_dtypes.bfloat16)

    # idx array [cores, 128, NBTOT*8] int16, 16-wrapped per call, replicated
    icol_base = 8 * call_block_base                                 # per call
    k_e = k_of[t, stream]
    i_in_call = gslot - call_block_base[k_e] * P
    col = icol_base[k_e] + (i_in_call >> 4)
    row16 = i_in_call & 15
    idx16 = np.zeros((NCORES, 16, NBTOT * 8), dtype=np.int16)
    idx16[core, row16, col] = pairrel
    idx_np = np.tile(idx16, (1, 8, 1))

    # CSR-ish offsets for degree (device computes deg = o2 - o1 + 1)
    o1_np = np.zeros((NCORES, P, NT), dtype=np.float32)
    o2_np = np.zeros((NCORES, P, NT), dtype=np.float32)
    for c in range(NCORES):
        cnt_node = np.bincount(dloc[core == c], minlength=NPAD).astype(np.int64)
        o2v = np.cumsum(cnt_node)
        o1v = o2v - cnt_node
        o1_np[c] = o1v.reshape(NT, P).T.astype(np.float32)
        o2_np[c] = o2v.reshape(NT, P).T.astype(np.float32)

    # Xt shards [64, 12544] fp32
    xt_np = np.zeros((NCORES, IN_C, NPAD), dtype=np.float32)
    for c in range(NCORES):
        xt_np[c, :, :NSH] = X[c * NSH:(c + 1) * NSH].T

    iota_np = np.broadcast_to(np.arange(P, dtype=np.float32), (P, P)).astype(ml_dtypes.bfloat16)
    ident_np = np.eye(P, dtype=np.float32).astype(ml_dtypes.bfloat16)
    b2rep_np = np.tile(b2.reshape(1, OUT_C), (P, TG)).astype(np.float32)  # [128, TG*64]

    # per-tile matmul emission metadata (shared across cores)
    tile_blocks = []  # tile t -> list of (call k, block-in-call b)
    for tt in range(NT):
        g = tt // TG
        lst = []
        for s in range(2):
            k = 2 * g + s
            for b in range(int(NB[tt, s])):
                lst.append((k, int(seg_block_base[tt, s]) + b))
        tile_blocks.append(lst)

    meta = dict(NB=NB, NBC=NBC.astype(int), NBTOT=NBTOT, tile_blocks=tile_blocks)
    per_core = []
    for c in range(NCORES):
        per_core.append({
            "xt": np.ascontiguousarray(xt_np[c]),
            "w1": W1, "b1": b1.reshape(HID, 1), "w2": W2, "b2": b2.reshape(1, OUT_C),
            "o1": np.ascontiguousarray(o1_np[c]), "o2": np.ascontiguousarray(o2_np[c]),
            "iota": iota_np, "ident": ident_np, "b2rep": b2rep_np,
            "dslot": np.ascontiguousarray(dslot_np[c]),
            "pmask": np.ascontiguousarray(pmask_np[c]),
            "idxs": np.ascontiguousarray(idx_np[c]),
        })
    return meta, per_core


def _build_program(meta):
    SKIP_GATHER = os.environ.get("SKIP_GATHER", "0") == "1"
    SKIP_S = os.environ.get("SKIP_S", "0") == "1"
    SKIP_SCAT = os.environ.get("SKIP_SCAT", "0") == "1"
    SKIP_AG = os.environ.get("SKIP_AG", "0") == "1"
    SKIP_DENSE = os.environ.get("SKIP_DENSE", "0") == "1"
    NBC = meta["NBC"]
    NBTOT = meta["NBTOT"]
    tile_blocks = meta["tile_blocks"]

    # split each call k into SPLITS sub-calls so several desc-gens (one per
    # SWDGE queue / Q7 core pair) run concurrently, with a deep buffer
    # rotation to pipeline desc-gen against drain + DVE + PE
    SPLITS = 3
    NSETS = 10
    OFF = []   # OFF[k][i] = block offset of sub-call i within old call k
    SZ = []
    for k in range(NCALL):
        n = int(NBC[k])
        sz = [(n + SPLITS - 1 - i) // SPLITS for i in range(SPLITS)]
        off = [0] * SPLITS
        for i in range(1, SPLITS):
            off[i] = off[i - 1] + sz[i - 1]
        OFF.append(off)
        SZ.append(sz)
    NCALL2 = SPLITS * NCALL
    NBC2 = [SZ[K // SPLITS][K % SPLITS] for K in range(NCALL2)]
    NBCMAX = max(NBC2)

    nc = bacc.Bacc("TRN2", num_devices=NCORES, num_swdge_queues=4)

    # --- DRAM tensors ---
    xt_d = nc.dram_tensor("xt", [IN_C, NPAD], FP32, kind="ExternalInput")
    w1_d = nc.dram_tensor("w1", [IN_C, HID], FP32, kind="ExternalInput")
    b1_d = nc.dram_tensor("b1", [HID, 1], FP32, kind="ExternalInput")
    w2_d = nc.dram_tensor("w2", [HID, OUT_C], FP32, kind="ExternalInput")
    o1_d = nc.dram_tensor("o1", [P, NT], FP32, kind="ExternalInput")
    o2_d = nc.dram_tensor("o2", [P, NT], FP32, kind="ExternalInput")
    iota_d = nc.dram_tensor("iota", [P, P], BF16, kind="ExternalInput")
    ident_d = nc.dram_tensor("ident", [P, P], BF16, kind="ExternalInput")
    b2rep_d = nc.dram_tensor("b2rep", [P, TG * OUT_C], FP32, kind="ExternalInput")
    dslot_d = nc.dram_tensor("dslot", [P, NBTOT], BF16, kind="ExternalInput")
    pmask_d = nc.dram_tensor("pmask", [P, NBTOT], BF16, kind="ExternalInput")
    idxs_d = nc.dram_tensor("idxs", [P, NBTOT * 8], I16, kind="ExternalInput")
    out_d = nc.dram_tensor("out", [NSH, OUT_C], FP32, kind="ExternalOutput")
    agin_d = nc.dram_tensor("agin", [NSH * OUT_C], BF16, kind="Internal")
    agout_d = nc.dram_tensor("agout", [PAIRS, P], BF16, kind="Internal",
                             addr_space="Shared")

    from contextlib import ExitStack
    stack = ExitStack()
    sem = {}
    for name in ["sin", "smm1", "srelu", "smm2", "sgn", "sagin", "scc", "sg",
                 "sS", "spe", "sactg", "sout", "swb", "scast", "sdeg", "sdinv",
                 "sbb", "sdense", "sdeg2", "sinc", "sidx", "sag1", "sag2"
                 ] + [f"sgs{i}" for i in range(12)]:
        sem[name] = stack.enter_context(nc.semaphore(name))

    # --- persistent SBUF ---
    sb = lambda name, shape, dt: stack.enter_context(nc.sbuf_tensor(name, shape, dt))
    w1_t = sb("w1t", [IN_C, HID], FP32)
    w1bf = sb("w1bf", [IN_C, HID], BF16)
    w2_t = sb("w2t", [HID, OUT_C], FP32)
    w2bf = sb("w2bf", [HID, OUT_C], BF16)
    b1_t = sb("b1t", [HID, 1], FP32)
    o1_t = sb("o1t", [P, NT], FP32)
    o2_t = sb("o2t", [P, NT], FP32)
    degm1 = sb("degm1", [P, NT], FP32)
    degp1 = sb("degp1", [P, NT], FP32)
    recip = sb("recip", [P, NT], FP32)
    dinv = sb("dinv", [P, NT], FP32)
    iota_t = sb("iotat", [P, P], BF16)
    ident_t = sb("identt", [P, P], BF16)
    b2rep_t = sb("b2rept", [P, TG * OUT_C], FP32)
    dslot_t = sb("dslott", [P, NBTOT], BF16)
    pmask_t = sb("pmaskt", [P, NBTOT], BF16)
    idx_t = sb("idxt", [P, NBTOT * 8], I16)
    g_node = sb("gnode", [P, NT, OUT_C], BF16)
    ostage = [sb(f"ost{i}", [P, TG * OUT_C], FP32) for i in range(2)]

    # PSUM: 8 banks exactly
    ps = lambda name, shape: stack.enter_context(nc.psum_tensor(name, shape, FP32))
    psum_h1 = [ps(f"ph1{i}", [P, 512]) for i in range(2)]
    psum_g = [ps(f"pg{i}", [P, 512]) for i in range(4)]
    psum_grp = [ps(f"pgrp{i}", [P, 512]) for i in range(2)]

    # dense-phase transient SBUF (freed before gather buffers are allocated)
    xt_ctx = nc.sbuf_tensor("xtf32", [IN_C, NPAD], FP32)
    xt_f32 = xt_ctx.__enter__()
    xtbf_ctx = nc.sbuf_tensor("xtbf", [IN_C, NPAD], BF16)
    xt_bf = xtbf_ctx.__enter__()
    htT_ctx = nc.sbuf_tensor("htT", [P, NPAD], BF16)
    htT = htT_ctx.__enter__()

    n_in = 0
    blk_cm = nc.Block()
    blk = blk_cm.__enter__()

    # ---------------- phase A ----------------
    inputs_list = [
        (xt_f32, xt_d), (w1_t, w1_d), (b1_t, b1_d), (w2_t, w2_d),
        (o1_t, o1_d), (o2_t, o2_d), (iota_t, iota_d), (ident_t, ident_d),
        (b2rep_t, b2rep_d),
        (dslot_t, dslot_d), (pmask_t, pmask_d), (idx_t, idxs_d),
    ]
    n_in = len(inputs_list)

    mm1_chunks = []
    c0 = 0
    while c0 < NPAD:
        w = min(512, NPAD - c0)
        mm1_chunks.append((c0, w))
        c0 += w
    NCH = len(mm1_chunks)

    # group-completion semaphores: a prefix wait on one shared counter is
    # unsound (increments from different in-flight DMAs interleave), so each
    # dependency group gets its own sem. Order matches inputs_list.
    input_sem_names = ["sdense"] * 4 + ["sdeg2"] * 2 + ["sinc"] * 5 + ["sidx"]

    GRP_HALF = 7                             # agin groups covering first half

    def emit_sp_a(sync):
        for (tile_sb, dram), sname in zip(inputs_list, input_sem_names):
            sync.dma_start(tile_sb[:], dram[:]).then_inc(sem[sname], 16)
        # write g shard to agin (node-major rows); first-half writes signal
        # sag1, second-half sag2 (AG1 must wait on a FULL group count — a
        # prefix wait on one shared counter is unsound with in-flight DMAs)
        nag1 = nag2 = 0
        for bidx in range(NGRP):
            t0 = bidx * TG
            t1 = min(t0 + TG, NT)
            full_t1 = min(t1, 97)  # tiles 0..96 are full
            hsem = "sag1" if bidx < GRP_HALF else "sag2"
            sync.wait_ge(sem["sgn"], min(t1, NT))
            if full_t1 > t0:
                dest = agin_d[t0 * P * OUT_C: full_t1 * P * OUT_C].rearrange(
                    "(t p d) -> p t d", p=P, t=full_t1 - t0)
                sync.dma_start(dest, g_node[:, t0:full_t1, :]).then_inc(sem[hsem], 16)
                if bidx < GRP_HALF:
                    nag1 += 1
                else:
                    nag2 += 1
            if t1 > 97:
                dest = agin_d[97 * P * OUT_C: NSH * OUT_C].rearrange(
                    "(p d) -> p d", p=LAST_ROWS)
                sync.dma_start(dest, g_node[:LAST_ROWS, 97, :]).then_inc(sem[hsem], 16)
                nag2 += 1
        emit_sp_a.nag1 = nag1
        emit_sp_a.nag2 = nag2

    blk.sync(emit_sp_a)
    NAG1 = emit_sp_a.nag1
    NAG2 = emit_sp_a.nag2

    def emit_dve_a(vector):
        vector.wait_ge(sem["sdense"], 16 * 4)   # xt, w1, b1, w2
        vector.tensor_copy(w1bf[:], w1_t[:])
        vector.tensor_copy(w2bf[:], w2_t[:])
        vector.nop().then_inc(sem["swb"], 1)
        vector.tensor_copy(xt_bf[:], xt_f32[:]).then_inc(sem["scast"], 1)
        vector.wait_ge(sem["sdeg2"], 16 * 2)    # o1, o2
        vector.tensor_tensor(out=degm1[:], in0=o2_t[:], in1=o1_t[:],
                             op=mybir.AluOpType.subtract)
        vector.tensor_scalar_add(degp1[:], degm1[:], 1.0)
        vector.reciprocal(recip[:], degp1[:]).then_inc(sem["sdeg"], 1)
        for j in range(NT):
            vector.wait_ge(sem["smm2"], j + 1)
            vector.tensor_scalar_mul(
                g_node[:, j, :], psum_g[j % 4][:, :OUT_C], dinv[:, j:j + 1]
            ).then_inc(sem["sgn"], 1)

    blk.vector(emit_dve_a)

    def emit_act_a(scalar):
        scalar.wait_ge(sem["sdense"], 16 * 4)
        scalar.wait_ge(sem["sdeg"], 1)
        scalar.activation(dinv[:], recip[:], mybir.ActivationFunctionType.Sqrt
                          ).then_inc(sem["sdinv"], 1)
        for ci, (cst, w) in enumerate(mm1_chunks):
            scalar.wait_ge(sem["smm1"], ci + 1)
            scalar.activation(htT[:, cst:cst + w], psum_h1[ci % 2][:, :w],
                              mybir.ActivationFunctionType.Relu,
                              bias=b1_t[:]).then_inc(sem["srelu"], 1)

    blk.scalar(emit_act_a)

    def emit_pe_a(tensor):
        tensor.wait_ge(sem["sdense"], 16 * 4)
        tensor.wait_ge(sem["swb"], 1)
        tensor.wait_ge(sem["scast"], 1)
        for ci, (cst, w) in enumerate(mm1_chunks):
            if ci >= 2:
                tensor.wait_ge(sem["srelu"], ci - 1)
            if SKIP_DENSE:
                tensor.nop().then_inc(sem["smm1"], 1)
                continue
            tensor.matmul(out=psum_h1[ci % 2][:, :w], lhsT=w1bf[:],
                          rhs=xt_bf[:, cst:cst + w], start=True,
                          stop=True).then_inc(sem["smm1"], 1)
        last_relu_wait = 0
        for j in range(NT):
            if j >= 4:
                tensor.wait_ge(sem["sgn"], j - 3)
            need = (j * P + P - 1) // 512 + 1
            if need > last_relu_wait:
                tensor.wait_ge(sem["srelu"], need)
                last_relu_wait = need
            tensor.matmul(out=psum_g[j % 4][:, :OUT_C],
                          lhsT=htT[:, j * P:(j + 1) * P], rhs=w2bf[:],
                          start=True, stop=True).then_inc(sem["smm2"], 1)

    blk.tensor(emit_pe_a)

    HALF_ELEMS = (NSH // 2) * OUT_C          # 6250*64 elems per half-shard
    GRP_HALF = 7                             # agin groups covering first half

    def emit_gp_a(gpsimd):
        gpsimd.load_library(mlp_library)
        if SKIP_AG:
            gpsimd.wait_ge(sem["sag1"], 16 * NAG1)
            gpsimd.wait_ge(sem["sag2"], 16 * NAG2)
            gpsimd.nop().then_inc(sem["scc"], 2)
        else:
            gpsimd.wait_ge(sem["sag1"], 16 * NAG1)
            gpsimd.collective_compute(
                "AllGather", mybir.AluOpType.bypass,
                replica_groups=[list(range(NCORES))],
                ins=[agin_d[:HALF_ELEMS].opt()],
                outs=[agout_d[:CHUNK, :].opt()],
            ).then_inc(sem["scc"], 1)
            gpsimd.wait_ge(sem["sag2"], 16 * NAG2)
            gpsimd.collective_compute(
                "AllGather", mybir.AluOpType.bypass,
                replica_groups=[list(range(NCORES))],
                ins=[agin_d[HALF_ELEMS:].opt()],
                outs=[agout_d[CHUNK:, :].opt()],
            ).then_inc(sem["scc"], 1)

    blk.gpsimd(emit_gp_a)

    # free dense transients, allocate gather buffers in their place
    htT_ctx.__exit__(None, None, None)
    xtbf_ctx.__exit__(None, None, None)
    xt_ctx.__exit__(None, None, None)

    gbuf = [stack.enter_context(nc.sbuf_tensor(f"gbuf{i}", [P, NBCMAX, P], BF16))
            for i in range(NSETS)]
    sbuf_S = [stack.enter_context(nc.sbuf_tensor(f"sS{i}", [P, NBCMAX, P], BF16))
              for i in range(NSETS)]
    gsel = [stack.enter_context(nc.sbuf_tensor(f"gsel{i}", [P, NBCMAX, OUT_C],
                                               BF16)) for i in range(NSETS)]

    # ---------------- phase B ----------------
    call_block_base = np.concatenate([[0], np.cumsum(NBC)])[:-1]

    def emit_gp_b(gpsimd):
        gpsimd.wait_ge(sem["sidx"], 16)         # idx table loaded
        # gbuf/sS/gsel reuse the SBUF freed from xt/htT: no gather DMA may
        # land until the dense phase has fully consumed those transients
        gpsimd.wait_ge(sem["sgn"], NT)
        for K in range(NCALL2):
            k, h = K // SPLITS, K % SPLITS
            c = k % 2
            nb = int(NBC2[K])
            if K < 2 * SPLITS:
                gpsimd.wait_ge(sem["scc"], c + 1)
            if nb == 0:
                gpsimd.nop().then_inc(sem[f"sgs{K % NSETS}"], 16)
                continue
            if K >= NSETS:
                gpsimd.wait_ge(sem["spe"], (K - NSETS) // (2 * SPLITS) + 1)
            b0 = int(call_block_base[k]) + OFF[k][h]
            colb = 8 * b0
            if SKIP_GATHER:
                gpsimd.nop().then_inc(sem[f"sgs{K % NSETS}"], 16)
            else:
                gpsimd.dma_gather(
                    gbuf[K % NSETS][:, :nb, :],
                    agout_d[c * CHUNK:(c + 1) * CHUNK, :],
                    idx_t[:, colb:colb + nb * 8],
                    nb * P, nb * P, P,
                    single_packet=False,
                    queue_num=K % 4,
                ).then_inc(sem[f"sgs{K % NSETS}"], 16)
        for s in range(NSETS):
            gpsimd.wait_ge(sem[f"sgs{s}"],
                           16 * ((NCALL2 - 1 - s) // NSETS + 1))

    def emit_dve_b(vector):
        vector.wait_ge(sem["sinc"], 16 * 5)     # iota/ident/b2rep/dslot/pmask

        def s_build(K):
            k, h = K // SPLITS, K % SPLITS
            nb = int(NBC2[K])
            if nb == 0:
                vector.nop().then_inc(sem["sS"], 1)
                return
            if K >= NSETS:
                vector.wait_ge(sem["spe"], (K - NSETS) // (2 * SPLITS) + 1)
            B0 = int(call_block_base[k]) + OFF[k][h]
            if SKIP_S:
                vector.nop().then_inc(sem["sS"], 1)
                return
            vector.tensor_tensor(
                out=sbuf_S[K % NSETS][:, :nb, :],
                in0=dslot_t[:, B0:B0 + nb, None].to_broadcast([P, nb, P]),
                in1=iota_t[:, None, :].to_broadcast([P, nb, P]),
                op=mybir.AluOpType.is_equal,
            )
            # parity select: gsel = lo + (hi - lo) * pmask. Per-SET DMA sem:
            # sets are strictly serialized by the spe recycle wait, so at most
            # one gather per set is in flight and the full-count wait is sound.
            vector.wait_ge(sem[f"sgs{K % NSETS}"], 16 * (K // NSETS + 1))
            vector.tensor_tensor(
                out=gsel[K % NSETS][:, :nb, :],
                in0=gbuf[K % NSETS][:, :nb, OUT_C:],
                in1=gbuf[K % NSETS][:, :nb, :OUT_C],
                op=mybir.AluOpType.subtract,
            )
            vector.tensor_tensor(
                out=gsel[K % NSETS][:, :nb, :],
                in0=gsel[K % NSETS][:, :nb, :],
                in1=pmask_t[:, B0:B0 + nb, None].to_broadcast([P, nb, OUT_C]),
                op=mybir.AluOpType.mult,
            )
            vector.tensor_tensor(
                out=gsel[K % NSETS][:, :nb, :],
                in0=gsel[K % NSETS][:, :nb, :],
                in1=gbuf[K % NSETS][:, :nb, :OUT_C],
                op=mybir.AluOpType.add,
            ).then_inc(sem["sS"], 1)

        def bias_add(g):
            vector.wait_ge(sem["sactg"], g + 1)
            vector.tensor_tensor(
                out=ostage[g % 2][:], in0=ostage[g % 2][:], in1=b2rep_t[:],
                op=mybir.AluOpType.add).then_inc(sem["sbb"], 1)

        for g in range(NGRP):
            for j in range(2 * SPLITS):
                s_build(2 * SPLITS * g + j)
            if g >= 2:
                bias_add(g - 2)
        bias_add(NGRP - 2)
        bias_add(NGRP - 1)

    def emit_pe_b(tensor):
        tensor.wait_ge(sem["sinc"], 16 * 5)     # ident loaded
        tensor.wait_ge(sem["sgn"], NT)
        for g in range(NGRP):
            if g >= 2:
                tensor.wait_ge(sem["sactg"], g - 1)
            tensor.wait_ge(sem["sS"], 2 * SPLITS * (g + 1))
            last = None
            for tl in range(TG):
                t = g * TG + tl
                pcol = psum_grp[g % 2][:, tl * OUT_C:(tl + 1) * OUT_C]
                first = True
                if not SKIP_SCAT:
                    for (k, b) in tile_blocks[t]:
                        h = 0
                        while h + 1 < SPLITS and b >= OFF[k][h + 1]:
                            h += 1
                        K = SPLITS * k + h
                        b2 = b - OFF[k][h]
                        last = tensor.matmul(
                            out=pcol,
                            lhsT=sbuf_S[K % NSETS][:, b2, :],
                            rhs=gsel[K % NSETS][:, b2, :],
                            start=first, stop=False)
                        first = False
                last = tensor.matmul(out=pcol, lhsT=ident_t[:],
                                     rhs=g_node[:, t, :], start=first, stop=True)
            last.then_inc(sem["spe"], 1)

    def emit_act_b(scalar):
        for g in range(NGRP):
            scalar.wait_ge(sem["spe"], g + 1)
            if g >= 2:
                scalar.wait_ge(sem["sout"], 16 * (g - 1))
            last = None
            for tl in range(TG):
                t = g * TG + tl
                rows = P if t < 97 else LAST_ROWS
                last = scalar.activation(
                    ostage[g % 2][:rows, tl * OUT_C:(tl + 1) * OUT_C],
                    psum_grp[g % 2][:rows, tl * OUT_C:(tl + 1) * OUT_C],
                    mybir.ActivationFunctionType.Copy,
                    scale=dinv[:rows, t:t + 1])
            last.then_inc(sem["sactg"], 1)

    def emit_sp_b(sync):
        nout = 0
        for g in range(NGRP):
            sync.wait_ge(sem["sbb"], g + 1)
            t0 = g * TG
            t1 = min(t0 + TG, NT)
            full_t1 = min(t1, 97)
            if full_t1 > t0:
                dest = out_d[t0 * P:full_t1 * P, :].rearrange(
                    "(t p) d -> p t d", p=P)
                sync.dma_start(dest, ostage[g % 2][:, : (full_t1 - t0) * OUT_C]
                               .rearrange("p (t d) -> p t d", d=OUT_C)
                               ).then_inc(sem["sout"], 16)
                nout += 1
            if t1 > 97:
                dest = out_d[97 * P:NSH, :]
                sync.dma_start(
                    dest,
                    ostage[g % 2][:LAST_ROWS, (97 - t0) * OUT_C:(98 - t0) * OUT_C],
                ).then_inc(sem["sout"], 16)
                nout += 1
        sync.wait_ge(sem["sout"], 16 * nout)

    blk.gpsimd(emit_gp_b)
    blk.vector(emit_dve_b)
    blk.tensor(emit_pe_b)
    blk.scalar(emit_act_b)
    blk.sync(emit_sp_b)

    blk_cm.__exit__(None, None, None)
    stack.close()
    nc.finalize()
    return nc


def kernel(node_features, edge_index, W1, b1, W2, b2):
    meta, per_core = _host_prep(node_features, edge_index, W1, b1, W2, b2)
    nc = _build_program(meta)
    trace = os.environ.get("GCN_TRACE", "0") == "1"
    res = run_bass_kernel_spmd(nc, per_core, core_ids=list(range(NCORES)),
                               trace=trace)
    LAST_EXEC_NS[0] = res.exec_time_ns
    LAST_TRACE[0] = res.instructions_and_trace
    out = np.concatenate([np.asarray(res.results[c]["out"]) for c in range(NCORES)],
                         axis=0)
    return out.astype(np.float32)



# revision 69
# speedup vs baseline: 1.1765x; 1.1765x over previous
"""GCNConv (fc+ReLU -> GCN propagate) distributed over 8 trn2 NeuronCores.

Strategy (graph/data parallel, per the sharding hint):
- Nodes sharded contiguously: core c owns nodes [c*12500, (c+1)*12500).
- Each core computes its shard of g = (relu(X@W1+b1) @ W2) * dinv on the
  tensor engine, then the g table (bf16) is AllGathered to every core's HBM.
- Edges are bucketed by dst shard (host side). Per core the edges are
  grouped by (dst tile of 128, src-table chunk, src parity) with
  program-uniform padded block counts so one SPMD program serves all cores.
- Messages are fetched with dma_gather (SDMA MoE gather) from the pair-packed
  table [50000, 128]bf16 (two nodes per 256B row, int16 chunk-relative
  indices), landing edge-major [128, nblk, 128]. The AllGather is split in
  two half-table collectives so chunk-0 gathers start earlier. Buckets are
  (dst tile, chunk) with a per-edge DVE parity select (lo + (hi-lo)*mask)
  picking the node half of each gathered pair row. Each bucket-group call is
  split into SPLITS sub-calls rotated over the 4 SWDGE queues (desc-gen for
  different queues overlaps on different Q7 core pairs; DMA completion is
  tracked with one semaphore per queue since cross-queue completions are
  unordered) and NSETS gbuf/S/gsel buffer sets pipeline desc-gen against
  drain + DVE + PE.
- Scatter-add runs on the tensor engine: per 128-edge block a one-hot
  S[edge, dstslot] (built on DVE via broadcast is_equal) multiplies the
  parity-selected block into the dst tile's PSUM accumulator. Self-loops are
  a per-tile identity matmul of the core's own g tile.
- Epilogue: ACT scales by dinv[dst] per partition, DVE adds b2, DMA out.
"""
import os
import numpy as np
import ml_dtypes

import concourse.bass as bass
import concourse.bacc as bacc
import concourse.mybir as mybir
from concourse.library_config import mlp as mlp_library
from concourse.bass_utils import run_bass_kernel_spmd

NCORES = 8
N = 100000
E_TOTAL = 1600000
IN_C = 64
HID = 128
OUT_C = 64
NSH = N // NCORES            # 12500 nodes per core
P = 128
NT = 98                      # dst tiles per core (97 full + 84)
NPAD = NT * P                # 12544 padded nodes per core
LAST_ROWS = NSH - 97 * P     # 84
PAIRS = N // 2               # 50000 pair rows
CHUNK = PAIRS // 2           # 25000 rows per gather chunk (int16 range)
TG = 7                       # tiles per group
NGRP = NT // TG              # 14 groups
NCALL = NGRP * 2             # 28 gather calls (group x chunk)

FP32 = mybir.dt.float32
BF16 = mybir.dt.bfloat16
I16 = mybir.dt.int16

LAST_EXEC_NS = [None]
LAST_TRACE = [None]


def _host_prep(node_features, edge_index, W1, b1, W2, b2):
    X = np.asarray(node_features, dtype=np.float32)
    ei = np.asarray(edge_index)
    W1 = np.asarray(W1, dtype=np.float32)
    b1 = np.asarray(b1, dtype=np.float32)
    W2 = np.asarray(W2, dtype=np.float32)
    b2 = np.asarray(b2, dtype=np.float32)
    src = ei[0].astype(np.int64)
    dst = ei[1].astype(np.int64)
    E = src.shape[0]

    core = dst // NSH
    dloc = dst - core * NSH
    t = dloc >> 7
    slot = dloc & 127
    # src table row mapping for the split AllGather: chunk h = which half of
    # the owning core's shard; row within chunk = src_core*3125 + local pair.
    scor = src // NSH
    sloc = src - scor * NSH
    half = (sloc >= (NSH // 2)).astype(np.int64)
    j = sloc - half * (NSH // 2)
    par = src & 1
    chunk = half
    stream = chunk                                # chunk only; parity via mask
    pairrel = (scor * (NSH // 4) + (j >> 1)).astype(np.int16)

    # per (core, t, stream) counts -> uniform block counts
    kf = ((core * NT + t) * 2 + stream)
    counts = np.bincount(kf, minlength=NCORES * NT * 2).reshape(NCORES, NT, 2)
    NB = np.maximum(np.ceil(counts.max(axis=0) / P).astype(np.int64), 0)  # [NT, 2]

    # call layout: call k = 2*g + c covers tiles 7g..7g+6, stream c
    NBC = np.zeros(NCALL, dtype=np.int64)
    seg_block_base = np.zeros((NT, 2), dtype=np.int64)  # block offset within call
    for g in range(NGRP):
        for c in range(2):
            k = 2 * g + c
            off = 0
            for tt in range(g * TG, (g + 1) * TG):
                seg_block_base[tt, c] = off
                off += NB[tt, c]
            NBC[k] = off
    NBTOT = int(NBC.sum())
    call_block_base = np.concatenate([[0], np.cumsum(NBC)])[:-1]   # blocks before call k

    # global slot position of each (t, s) segment
    k_of = np.zeros((NT, 2), dtype=np.int64)
    for tt in range(NT):
        for s in range(2):
            k_of[tt, s] = 2 * (tt // TG) + s
    GPOS = (call_block_base[k_of] + seg_block_base) * P            # [NT, 2]

    # rank of each edge within its (core, t, s) bucket
    order = np.argsort(kf, kind="stable")
    cnt_flat = np.bincount(kf, minlength=NCORES * NT * 2)
    start_flat = np.concatenate([[0], np.cumsum(cnt_flat)])[:-1]
    rank = np.empty(E, dtype=np.int64)
    rank[order] = np.arange(E) - start_flat[kf[order]]

    gslot = GPOS[t, stream] + rank                                  # [E]
    Bg = gslot >> 7
    prow = gslot & 127

    # dstslot array [cores, 128, NBTOT] bf16 (pad = -1)
    dslot_np = np.full((NCORES, P, NBTOT), -1.0, dtype=np.float32)
    dslot_np[core, prow, Bg] = slot
    dslot_np = dslot_np.astype(ml_dtypes.bfloat16)

    # parity mask [cores, 128, NBTOT] bf16 (1 -> use odd half of pair row)
    pmask_np = np.zeros((NCORES, P, NBTOT), dtype=np.float32)
    pmask_np[core, prow, Bg] = par
    pmask_np = pmask_np.astype(ml_dtypes.bfloat16)

    # idx array [cores, 128, NBTOT*8] int16, 16-wrapped per call, replicated
    icol_base = 8 * call_block_base                                 # per call
    k_e = k_of[t, stream]
    i_in_call = gslot - call_block_base[k_e] * P
    col = icol_base[k_e] + (i_in_call >> 4)
    row16 = i_in_call & 15
    idx16 = np.zeros((NCORES, 16, NBTOT * 8), dtype=np.int16)
    idx16[core, row16, col] = pairrel
    idx_np = np.tile(idx16, (1, 8, 1))

    # CSR-ish offsets for degree (device computes deg = o2 - o1 + 1)
    o1_np = np.zeros((NCORES, P, NT), dtype=np.float32)
    o2_np = np.zeros((NCORES, P, NT), dtype=np.float32)
    for c in range(NCORES):
        cnt_node = np.bincount(dloc[core == c], minlength=NPAD).astype(np.int64)
        o2v = np.cumsum(cnt_node)
        o1v = o2v - cnt_node
        o1_np[c] = o1v.reshape(NT, P).T.astype(np.float32)
        o2_np[c] = o2v.reshape(NT, P).T.astype(np.float32)

    # Xt shards [64, 12544] fp32
    xt_np = np.zeros((NCORES, IN_C, NPAD), dtype=np.float32)
    for c in range(NCORES):
        xt_np[c, :, :NSH] = X[c * NSH:(c + 1) * NSH].T

    iota_np = np.broadcast_to(np.arange(P, dtype=np.float32), (P, P)).astype(ml_dtypes.bfloat16)
    ident_np = np.eye(P, dtype=np.float32).astype(ml_dtypes.bfloat16)
    b2rep_np = np.tile(b2.reshape(1, OUT_C), (P, TG)).astype(np.float32)  # [128, TG*64]

    # per-tile matmul emission metadata (shared across cores)
    tile_blocks = []  # tile t -> list of (call k, block-in-call b)
    for tt in range(NT):
        g = tt // TG
        lst = []
        for s in range(2):
            k = 2 * g + s
            for b in range(int(NB[tt, s])):
                lst.append((k, int(seg_block_base[tt, s]) + b))
        tile_blocks.append(lst)

    meta = dict(NB=NB, NBC=NBC.astype(int), NBTOT=NBTOT, tile_blocks=tile_blocks)
    per_core = []
    for c in range(NCORES):
        per_core.append({
            "xt": np.ascontiguousarray(xt_np[c]),
            "w1": W1, "b1": b1.reshape(HID, 1), "w2": W2, "b2": b2.reshape(1, OUT_C),
            "o1": np.ascontiguousarray(o1_np[c]), "o2": np.ascontiguousarray(o2_np[c]),
            "iota": iota_np, "ident": ident_np, "b2rep": b2rep_np,
            "dslot": np.ascontiguousarray(dslot_np[c]),
            "pmask": np.ascontiguousarray(pmask_np[c]),
            "idxs": np.ascontiguousarray(idx_np[c]),
        })
    return meta, per_core


def _build_program(meta):
    SKIP_GATHER = os.environ.get("SKIP_GATHER", "0") == "1"
    SKIP_S = os.environ.get("SKIP_S", "0") == "1"
    SKIP_SCAT = os.environ.get("SKIP_SCAT", "0") == "1"
    SKIP_AG = os.environ.get("SKIP_AG", "0") == "1"
    SKIP_DENSE = os.environ.get("SKIP_DENSE", "0") == "1"
    NBC = meta["NBC"]
    NBTOT = meta["NBTOT"]
    tile_blocks = meta["tile_blocks"]

    # split each call k into SPLITS sub-calls so several desc-gens (one per
    # SWDGE queue / Q7 core pair) run concurrently, with a deep buffer
    # rotation to pipeline desc-gen against drain + DVE + PE
    SPLITS = 3
    NSETS = 12          # exactly two 6-call batches -> full double buffering
    NSETS_EARLY = 5     # sets allocated outside the dense-transient region
    OFF = []   # OFF[k][i] = block offset of sub-call i within old call k
    SZ = []
    for k in range(NCALL):
        n = int(NBC[k])
        sz = [(n + SPLITS - 1 - i) // SPLITS for i in range(SPLITS)]
        off = [0] * SPLITS
        for i in range(1, SPLITS):
            off[i] = off[i - 1] + sz[i - 1]
        OFF.append(off)
        SZ.append(sz)
    NCALL2 = SPLITS * NCALL
    NBC2 = [SZ[K // SPLITS][K % SPLITS] for K in range(NCALL2)]
    NBCMAX = max(NBC2)

    nc = bacc.Bacc("TRN2", num_devices=NCORES, num_swdge_queues=4)

    # --- DRAM tensors ---
    xt_d = nc.dram_tensor("xt", [IN_C, NPAD], FP32, kind="ExternalInput")
    w1_d = nc.dram_tensor("w1", [IN_C, HID], FP32, kind="ExternalInput")
    b1_d = nc.dram_tensor("b1", [HID, 1], FP32, kind="ExternalInput")
    w2_d = nc.dram_tensor("w2", [HID, OUT_C], FP32, kind="ExternalInput")
    o1_d = nc.dram_tensor("o1", [P, NT], FP32, kind="ExternalInput")
    o2_d = nc.dram_tensor("o2", [P, NT], FP32, kind="ExternalInput")
    iota_d = nc.dram_tensor("iota", [P, P], BF16, kind="ExternalInput")
    ident_d = nc.dram_tensor("ident", [P, P], BF16, kind="ExternalInput")
    b2rep_d = nc.dram_tensor("b2rep", [P, TG * OUT_C], FP32, kind="ExternalInput")
    dslot_d = nc.dram_tensor("dslot", [P, NBTOT], BF16, kind="ExternalInput")
    pmask_d = nc.dram_tensor("pmask", [P, NBTOT], BF16, kind="ExternalInput")
    idxs_d = nc.dram_tensor("idxs", [P, NBTOT * 8], I16, kind="ExternalInput")
    out_d = nc.dram_tensor("out", [NSH, OUT_C], FP32, kind="ExternalOutput")
    agin_d = nc.dram_tensor("agin", [NSH * OUT_C], BF16, kind="Internal")
    agout_d = nc.dram_tensor("agout", [PAIRS, P], BF16, kind="Internal",
                             addr_space="Shared")

    from contextlib import ExitStack
    stack = ExitStack()
    sem = {}
    for name in ["sin", "smm1", "srelu", "smm2", "sgn", "sagin", "scc", "sg",
                 "sS", "spe", "sactg", "sout", "swb", "scast", "sdeg", "sdinv",
                 "sbb", "sdense", "sdeg2", "sinc", "sidx", "sag1", "sag2"
                 ] + [f"sgs{i}" for i in range(16)]:
        sem[name] = stack.enter_context(nc.semaphore(name))

    # --- persistent SBUF ---
    sb = lambda name, shape, dt: stack.enter_context(nc.sbuf_tensor(name, shape, dt))
    w1_t = sb("w1t", [IN_C, HID], FP32)
    w1bf = sb("w1bf", [IN_C, HID], BF16)
    w2_t = sb("w2t", [HID, OUT_C], FP32)
    w2bf = sb("w2bf", [HID, OUT_C], BF16)
    b1_t = sb("b1t", [HID, 1], FP32)
    o1_t = sb("o1t", [P, NT], FP32)
    o2_t = sb("o2t", [P, NT], FP32)
    degm1 = sb("degm1", [P, NT], FP32)
    degp1 = sb("degp1", [P, NT], FP32)
    recip = sb("recip", [P, NT], FP32)
    dinv = sb("dinv", [P, NT], FP32)
    iota_t = sb("iotat", [P, P], BF16)
    ident_t = sb("identt", [P, P], BF16)
    b2rep_t = sb("b2rept", [P, TG * OUT_C], FP32)
    dslot_t = sb("dslott", [P, NBTOT], BF16)
    pmask_t = sb("pmaskt", [P, NBTOT], BF16)
    idx_t = sb("idxt", [P, NBTOT * 8], I16)
    g_node = sb("gnode", [P, NT, OUT_C], BF16)
    ostage = [sb(f"ost{i}", [P, TG * OUT_C], FP32) for i in range(2)]

    # PSUM: 8 banks exactly
    ps = lambda name, shape: stack.enter_context(nc.psum_tensor(name, shape, FP32))
    psum_h1 = [ps(f"ph1{i}", [P, 512]) for i in range(2)]
    psum_g = [ps(f"pg{i}", [P, 512]) for i in range(4)]
    psum_grp = [ps(f"pgrp{i}", [P, 512]) for i in range(2)]

    # the first NSETS_EARLY buffer sets are allocated below the dense
    # transients (stack allocator), so they never overlap them: their gather
    # DMAs may fire before the dense phase finishes
    gbuf = [stack.enter_context(nc.sbuf_tensor(f"gbuf{i}", [P, NBCMAX, P], BF16))
            for i in range(NSETS_EARLY)]
    sbuf_S = [stack.enter_context(nc.sbuf_tensor(f"sS{i}", [P, NBCMAX, P], BF16))
              for i in range(NSETS_EARLY)]

    # dense-phase transient SBUF (freed before gather buffers are allocated)
    xt_ctx = nc.sbuf_tensor("xtf32", [IN_C, NPAD], FP32)
    xt_f32 = xt_ctx.__enter__()
    xtbf_ctx = nc.sbuf_tensor("xtbf", [IN_C, NPAD], BF16)
    xt_bf = xtbf_ctx.__enter__()
    htT_ctx = nc.sbuf_tensor("htT", [P, NPAD], BF16)
    htT = htT_ctx.__enter__()

    n_in = 0
    blk_cm = nc.Block()
    blk = blk_cm.__enter__()

    # ---------------- phase A ----------------
    inputs_list = [
        (xt_f32, xt_d), (w1_t, w1_d), (b1_t, b1_d), (w2_t, w2_d),
        (o1_t, o1_d), (o2_t, o2_d), (iota_t, iota_d), (ident_t, ident_d),
        (b2rep_t, b2rep_d),
        (dslot_t, dslot_d), (pmask_t, pmask_d), (idx_t, idxs_d),
    ]
    n_in = len(inputs_list)

    mm1_chunks = []
    c0 = 0
    while c0 < NPAD:
        w = min(512, NPAD - c0)
        mm1_chunks.append((c0, w))
        c0 += w
    NCH = len(mm1_chunks)

    # group-completion semaphores: a prefix wait on one shared counter is
    # unsound (increments from different in-flight DMAs interleave), so each
    # dependency group gets its own sem. Order matches inputs_list.
    input_sem_names = ["sdense"] * 4 + ["sdeg2"] * 2 + ["sinc"] * 5 + ["sidx"]

    GRP_HALF = 7                             # agin groups covering first half

    def emit_sp_a(sync):
        for (tile_sb, dram), sname in zip(inputs_list, input_sem_names):
            sync.dma_start(tile_sb[:], dram[:]).then_inc(sem[sname], 16)
        # write g shard to agin (node-major rows); first-half writes signal
        # sag1, second-half sag2 (AG1 must wait on a FULL group count — a
        # prefix wait on one shared counter is unsound with in-flight DMAs)
        nag1 = nag2 = 0
        for bidx in range(NGRP):
            t0 = bidx * TG
            t1 = min(t0 + TG, NT)
            full_t1 = min(t1, 97)  # tiles 0..96 are full
            hsem = "sag1" if bidx < GRP_HALF else "sag2"
            sync.wait_ge(sem["sgn"], min(t1, NT))
            if full_t1 > t0:
                dest = agin_d[t0 * P * OUT_C: full_t1 * P * OUT_C].rearrange(
                    "(t p d) -> p t d", p=P, t=full_t1 - t0)
                sync.dma_start(dest, g_node[:, t0:full_t1, :]).then_inc(sem[hsem], 16)
                if bidx < GRP_HALF:
                    nag1 += 1
                else:
                    nag2 += 1
            if t1 > 97:
                dest = agin_d[97 * P * OUT_C: NSH * OUT_C].rearrange(
                    "(p d) -> p d", p=LAST_ROWS)
                sync.dma_start(dest, g_node[:LAST_ROWS, 97, :]).then_inc(sem[hsem], 16)
                nag2 += 1
        emit_sp_a.nag1 = nag1
        emit_sp_a.nag2 = nag2

    blk.sync(emit_sp_a)
    NAG1 = emit_sp_a.nag1
    NAG2 = emit_sp_a.nag2

    def emit_dve_a(vector):
        vector.wait_ge(sem["sdense"], 16 * 4)   # xt, w1, b1, w2
        vector.tensor_copy(w1bf[:], w1_t[:])
        vector.tensor_copy(w2bf[:], w2_t[:])
        vector.nop().then_inc(sem["swb"], 1)
        vector.tensor_copy(xt_bf[:], xt_f32[:]).then_inc(sem["scast"], 1)
        vector.wait_ge(sem["sdeg2"], 16 * 2)    # o1, o2
        vector.tensor_tensor(out=degm1[:], in0=o2_t[:], in1=o1_t[:],
                             op=mybir.AluOpType.subtract)
        vector.tensor_scalar_add(degp1[:], degm1[:], 1.0)
        vector.reciprocal(recip[:], degp1[:]).then_inc(sem["sdeg"], 1)
        for j in range(NT):
            vector.wait_ge(sem["smm2"], j + 1)
            vector.tensor_scalar_mul(
                g_node[:, j, :], psum_g[j % 4][:, :OUT_C], dinv[:, j:j + 1]
            ).then_inc(sem["sgn"], 1)

    blk.vector(emit_dve_a)

    def emit_act_a(scalar):
        scalar.wait_ge(sem["sdense"], 16 * 4)
        scalar.wait_ge(sem["sdeg"], 1)
        scalar.activation(dinv[:], recip[:], mybir.ActivationFunctionType.Sqrt
                          ).then_inc(sem["sdinv"], 1)
        for ci, (cst, w) in enumerate(mm1_chunks):
            scalar.wait_ge(sem["smm1"], ci + 1)
            scalar.activation(htT[:, cst:cst + w], psum_h1[ci % 2][:, :w],
                              mybir.ActivationFunctionType.Relu,
                              bias=b1_t[:]).then_inc(sem["srelu"], 1)

    blk.scalar(emit_act_a)

    def emit_pe_a(tensor):
        tensor.wait_ge(sem["sdense"], 16 * 4)
        tensor.wait_ge(sem["swb"], 1)
        tensor.wait_ge(sem["scast"], 1)
        for ci, (cst, w) in enumerate(mm1_chunks):
            if ci >= 2:
                tensor.wait_ge(sem["srelu"], ci - 1)
            if SKIP_DENSE:
                tensor.nop().then_inc(sem["smm1"], 1)
                continue
            tensor.matmul(out=psum_h1[ci % 2][:, :w], lhsT=w1bf[:],
                          rhs=xt_bf[:, cst:cst + w], start=True,
                          stop=True).then_inc(sem["smm1"], 1)
        last_relu_wait = 0
        for j in range(NT):
            if j >= 4:
                tensor.wait_ge(sem["sgn"], j - 3)
            need = (j * P + P - 1) // 512 + 1
            if need > last_relu_wait:
                tensor.wait_ge(sem["srelu"], need)
                last_relu_wait = need
            tensor.matmul(out=psum_g[j % 4][:, :OUT_C],
                          lhsT=htT[:, j * P:(j + 1) * P], rhs=w2bf[:],
                          start=True, stop=True).then_inc(sem["smm2"], 1)

    blk.tensor(emit_pe_a)

    HALF_ELEMS = (NSH // 2) * OUT_C          # 6250*64 elems per half-shard
    GRP_HALF = 7                             # agin groups covering first half

    def emit_gp_a(gpsimd):
        gpsimd.load_library(mlp_library)
        if SKIP_AG:
            gpsimd.wait_ge(sem["sag1"], 16 * NAG1)
            gpsimd.wait_ge(sem["sag2"], 16 * NAG2)
            gpsimd.nop().then_inc(sem["scc"], 2)
        else:
            gpsimd.wait_ge(sem["sag1"], 16 * NAG1)
            gpsimd.collective_compute(
                "AllGather", mybir.AluOpType.bypass,
                replica_groups=[list(range(NCORES))],
                ins=[agin_d[:HALF_ELEMS].opt()],
                outs=[agout_d[:CHUNK, :].opt()],
            ).then_inc(sem["scc"], 1)
            gpsimd.wait_ge(sem["sag2"], 16 * NAG2)
            gpsimd.collective_compute(
                "AllGather", mybir.AluOpType.bypass,
                replica_groups=[list(range(NCORES))],
                ins=[agin_d[HALF_ELEMS:].opt()],
                outs=[agout_d[CHUNK:, :].opt()],
            ).then_inc(sem["scc"], 1)

    blk.gpsimd(emit_gp_a)

    # free dense transients, allocate the remaining sets in their place
    htT_ctx.__exit__(None, None, None)
    xtbf_ctx.__exit__(None, None, None)
    xt_ctx.__exit__(None, None, None)

    gbuf += [stack.enter_context(nc.sbuf_tensor(f"gbuf{i}", [P, NBCMAX, P], BF16))
             for i in range(NSETS_EARLY, NSETS)]
    sbuf_S += [stack.enter_context(nc.sbuf_tensor(f"sS{i}", [P, NBCMAX, P], BF16))
               for i in range(NSETS_EARLY, NSETS)]

    # ---------------- phase B ----------------
    call_block_base = np.concatenate([[0], np.cumsum(NBC)])[:-1]

    def emit_gp_b(gpsimd):
        gpsimd.wait_ge(sem["sidx"], 16)         # idx table loaded
        for K in range(NCALL2):
            if K == NSETS_EARLY:
                # sets >= NSETS_EARLY reuse the SBUF freed from xt/htT: their
                # gather DMAs may not land until the dense phase has fully
                # consumed those transients
                gpsimd.wait_ge(sem["sgn"], NT)
            k, h = K // SPLITS, K % SPLITS
            c = k % 2
            nb = int(NBC2[K])
            if K < 2 * SPLITS:
                gpsimd.wait_ge(sem["scc"], c + 1)
            if nb == 0:
                gpsimd.nop().then_inc(sem[f"sgs{K % NSETS}"], 16)
                continue
            if K >= NSETS:
                gpsimd.wait_ge(sem["spe"], (K - NSETS) // (2 * SPLITS) + 1)
            b0 = int(call_block_base[k]) + OFF[k][h]
            colb = 8 * b0
            if SKIP_GATHER:
                gpsimd.nop().then_inc(sem[f"sgs{K % NSETS}"], 16)
            else:
                gpsimd.dma_gather(
                    gbuf[K % NSETS][:, :nb, :],
                    agout_d[c * CHUNK:(c + 1) * CHUNK, :],
                    idx_t[:, colb:colb + nb * 8],
                    nb * P, nb * P, P,
                    single_packet=False,
                    queue_num=K % 4,
                ).then_inc(sem[f"sgs{K % NSETS}"], 16)
        for s in range(NSETS):
            gpsimd.wait_ge(sem[f"sgs{s}"],
                           16 * ((NCALL2 - 1 - s) // NSETS + 1))

    def emit_dve_b(vector):
        vector.wait_ge(sem["sinc"], 16 * 5)     # iota/ident/b2rep/dslot/pmask

        def s_build(K):
            k, h = K // SPLITS, K % SPLITS
            nb = int(NBC2[K])
            if nb == 0:
                vector.nop().then_inc(sem["sS"], 1)
                return
            if K >= NSETS:
                vector.wait_ge(sem["spe"], (K - NSETS) // (2 * SPLITS) + 1)
            B0 = int(call_block_base[k]) + OFF[k][h]
            if SKIP_S:
                vector.nop().then_inc(sem["sS"], 1)
                return
            vector.tensor_tensor(
                out=sbuf_S[K % NSETS][:, :nb, :],
                in0=dslot_t[:, B0:B0 + nb, None].to_broadcast([P, nb, P]),
                in1=iota_t[:, None, :].to_broadcast([P, nb, P]),
                op=mybir.AluOpType.is_equal,
            )
            # in-place parity select: lo += (hi - lo) * pmask (hi half used
            # as scratch; result lands in the lo half). Per-SET DMA sem: sets
            # are strictly serialized by the spe recycle wait, so at most one
            # gather per set is in flight and the full-count wait is sound.
            g = gbuf[K % NSETS]
            vector.wait_ge(sem[f"sgs{K % NSETS}"], 16 * (K // NSETS + 1))
            vector.tensor_tensor(
                out=g[:, :nb, OUT_C:],
                in0=g[:, :nb, OUT_C:],
                in1=g[:, :nb, :OUT_C],
                op=mybir.AluOpType.subtract,
            )
            vector.tensor_tensor(
                out=g[:, :nb, OUT_C:],
                in0=g[:, :nb, OUT_C:],
                in1=pmask_t[:, B0:B0 + nb, None].to_broadcast([P, nb, OUT_C]),
                op=mybir.AluOpType.mult,
            )
            vector.tensor_tensor(
                out=g[:, :nb, :OUT_C],
                in0=g[:, :nb, :OUT_C],
                in1=g[:, :nb, OUT_C:],
                op=mybir.AluOpType.add,
            ).then_inc(sem["sS"], 1)

        def bias_add(g):
            vector.wait_ge(sem["sactg"], g + 1)
            vector.tensor_tensor(
                out=ostage[g % 2][:], in0=ostage[g % 2][:], in1=b2rep_t[:],
                op=mybir.AluOpType.add).then_inc(sem["sbb"], 1)

        for g in range(NGRP):
            for j in range(2 * SPLITS):
                s_build(2 * SPLITS * g + j)
            if g >= 2:
                bias_add(g - 2)
        bias_add(NGRP - 2)
        bias_add(NGRP - 1)

    def emit_pe_b(tensor):
        tensor.wait_ge(sem["sinc"], 16 * 5)     # ident loaded
        tensor.wait_ge(sem["sgn"], NT)
        for g in range(NGRP):
            if g >= 2:
                tensor.wait_ge(sem["sactg"], g - 1)
            tensor.wait_ge(sem["sS"], 2 * SPLITS * (g + 1))
            last = None
            for tl in range(TG):
                t = g * TG + tl
                pcol = psum_grp[g % 2][:, tl * OUT_C:(tl + 1) * OUT_C]
                first = True
                if not SKIP_SCAT:
                    for (k, b) in tile_blocks[t]:
                        h = 0
                        while h + 1 < SPLITS and b >= OFF[k][h + 1]:
                            h += 1
                        K = SPLITS * k + h
                        b2 = b - OFF[k][h]
                        last = tensor.matmul(
                            out=pcol,
                            lhsT=sbuf_S[K % NSETS][:, b2, :],
                            rhs=gbuf[K % NSETS][:, b2, :OUT_C],
                            start=first, stop=False)
                        first = False
                last = tensor.matmul(out=pcol, lhsT=ident_t[:],
                                     rhs=g_node[:, t, :], start=first, stop=True)
            last.then_inc(sem["spe"], 1)

    def emit_act_b(scalar):
        for g in range(NGRP):
            scalar.wait_ge(sem["spe"], g + 1)
            if g >= 2:
                scalar.wait_ge(sem["sout"], 16 * (g - 1))
            last = None
            for tl in range(TG):
                t = g * TG + tl
                rows = P if t < 97 else LAST_ROWS
                last = scalar.activation(
                    ostage[g % 2][:rows, tl * OUT_C:(tl + 1) * OUT_C],
                    psum_grp[g % 2][:rows, tl * OUT_C:(tl + 1) * OUT_C],
                    mybir.ActivationFunctionType.Copy,
                    scale=dinv[:rows, t:t + 1])
            last.then_inc(sem["sactg"], 1)

    def emit_sp_b(sync):
        nout = 0
        for g in range(NGRP):
            sync.wait_ge(sem["sbb"], g + 1)
            t0 = g * TG
            t1 = min(t0 + TG, NT)
            full_t1 = min(t1, 97)
            if full_t1 > t0:
                dest = out_d[t0 * P:full_t1 * P, :].rearrange(
                    "(t p) d -> p t d", p=P)
                sync.dma_start(dest, ostage[g % 2][:, : (full_t1 - t0) * OUT_C]
                               .rearrange("p (t d) -> p t d", d=OUT_C)
                               ).then_inc(sem["sout"], 16)
                nout += 1
            if t1 > 97:
                dest = out_d[97 * P:NSH, :]
                sync.dma_start(
                    dest,
                    ostage[g % 2][:LAST_ROWS, (97 - t0) * OUT_C:(98 - t0) * OUT_C],
                ).then_inc(sem["sout"], 16)
                nout += 1
        sync.wait_ge(sem["sout"], 16 * nout)

    blk.gpsimd(emit_gp_b)
    blk.vector(emit_dve_b)
    blk.tensor(emit_pe_b)
    blk.scalar(emit_act_b)
    blk.sync(emit_sp_b)

    blk_cm.__exit__(None, None, None)
    stack.close()
    nc.finalize()
    return nc


def kernel(node_features, edge_index, W1, b1, W2, b2):
    meta, per_core = _host_prep(node_features, edge_index, W1, b1, W2, b2)
    nc = _build_program(meta)
    trace = os.environ.get("GCN_TRACE", "0") == "1"
    res = run_bass_kernel_spmd(nc, per_core, core_ids=list(range(NCORES)),
                               trace=trace)
    LAST_EXEC_NS[0] = res.exec_time_ns
    LAST_TRACE[0] = res.instructions_and_trace
    out = np.concatenate([np.asarray(res.results[c]["out"]) for c in range(NCORES)],
                         axis=0)
    return out.astype(np.float32)



# revision 72
# speedup vs baseline: 1.1912x; 1.0125x over previous
"""GCNConv (fc+ReLU -> GCN propagate) distributed over 8 trn2 NeuronCores.

Strategy (graph/data parallel, per the sharding hint):
- Nodes sharded contiguously: core c owns nodes [c*12500, (c+1)*12500).
- Each core computes its shard of g = (relu(X@W1+b1) @ W2) * dinv on the
  tensor engine, then the g table (bf16) is AllGathered to every core's HBM.
- Edges are bucketed by dst shard (host side). Per core the edges are
  grouped by (dst tile of 128, src-table chunk, src parity) with
  program-uniform padded block counts so one SPMD program serves all cores.
- Messages are fetched with dma_gather (SDMA MoE gather) from the pair-packed
  table [50000, 128]bf16 (two nodes per 256B row, int16 chunk-relative
  indices), landing edge-major [128, nblk, 128]. The AllGather is split in
  two half-table collectives so chunk-0 gathers start earlier. Buckets are
  (dst tile, chunk) with a per-edge DVE parity select (lo + (hi-lo)*mask)
  picking the node half of each gathered pair row. Each bucket-group call is
  split into SPLITS sub-calls rotated over the 4 SWDGE queues (desc-gen for
  different queues overlaps on different Q7 core pairs; DMA completion is
  tracked with one semaphore per queue since cross-queue completions are
  unordered) and NSETS gbuf/S buffer sets (parity select done in place in
  gbuf) pipeline desc-gen against drain + DVE + PE; the first NSETS_EARLY
  sets sit below the dense transients so their gathers skip the dense gate.
- Scatter-add runs on the tensor engine: per 128-edge block a one-hot
  S[edge, dstslot] (built on DVE via broadcast is_equal) multiplies the
  parity-selected block into the dst tile's PSUM accumulator. Self-loops are
  a per-tile identity matmul of the core's own g tile.
- Epilogue: ACT scales by dinv[dst] per partition, DVE adds b2, DMA out.
"""
import os
import numpy as np
import ml_dtypes

import concourse.bass as bass
import concourse.bacc as bacc
import concourse.mybir as mybir
from concourse.library_config import mlp as mlp_library
from concourse.bass_utils import run_bass_kernel_spmd

NCORES = 8
N = 100000
E_TOTAL = 1600000
IN_C = 64
HID = 128
OUT_C = 64
NSH = N // NCORES            # 12500 nodes per core
P = 128
NT = 98                      # dst tiles per core (97 full + 84)
NPAD = NT * P                # 12544 padded nodes per core
LAST_ROWS = NSH - 97 * P     # 84
PAIRS = N // 2               # 50000 pair rows
CHUNK = PAIRS // 2           # 25000 rows per gather chunk (int16 range)
TG = 7                       # tiles per group
NGRP = NT // TG              # 14 groups
NCALL = NGRP * 2             # 28 gather calls (group x chunk)

FP32 = mybir.dt.float32
BF16 = mybir.dt.bfloat16
I16 = mybir.dt.int16

LAST_EXEC_NS = [None]
LAST_TRACE = [None]


def _host_prep(node_features, edge_index, W1, b1, W2, b2):
    X = np.asarray(node_features, dtype=np.float32)
    ei = np.asarray(edge_index)
    W1 = np.asarray(W1, dtype=np.float32)
    b1 = np.asarray(b1, dtype=np.float32)
    W2 = np.asarray(W2, dtype=np.float32)
    b2 = np.asarray(b2, dtype=np.float32)
    src = ei[0].astype(np.int64)
    dst = ei[1].astype(np.int64)
    E = src.shape[0]

    core = dst // NSH
    dloc = dst - core * NSH
    t = dloc >> 7
    slot = dloc & 127
    # src table row mapping for the split AllGather: chunk h = which half of
    # the owning core's shard; row within chunk = src_core*3125 + local pair.
    scor = src // NSH
    sloc = src - scor * NSH
    half = (sloc >= (NSH // 2)).astype(np.int64)
    j = sloc - half * (NSH // 2)
    par = src & 1
    chunk = half
    stream = chunk                                # chunk only; parity via mask
    pairrel = (scor * (NSH // 4) + (j >> 1)).astype(np.int16)

    # per (core, t, stream) counts -> uniform block counts
    kf = ((core * NT + t) * 2 + stream)
    counts = np.bincount(kf, minlength=NCORES * NT * 2).reshape(NCORES, NT, 2)
    NB = np.maximum(np.ceil(counts.max(axis=0) / P).astype(np.int64), 0)  # [NT, 2]

    # call layout: call k = 2*g + c covers tiles 7g..7g+6, stream c
    NBC = np.zeros(NCALL, dtype=np.int64)
    seg_block_base = np.zeros((NT, 2), dtype=np.int64)  # block offset within call
    for g in range(NGRP):
        for c in range(2):
            k = 2 * g + c
            off = 0
            for tt in range(g * TG, (g + 1) * TG):
                seg_block_base[tt, c] = off
                off += NB[tt, c]
            NBC[k] = off
    NBTOT = int(NBC.sum())
    call_block_base = np.concatenate([[0], np.cumsum(NBC)])[:-1]   # blocks before call k

    # global slot position of each (t, s) segment
    k_of = np.zeros((NT, 2), dtype=np.int64)
    for tt in range(NT):
        for s in range(2):
            k_of[tt, s] = 2 * (tt // TG) + s
    GPOS = (call_block_base[k_of] + seg_block_base) * P            # [NT, 2]

    # rank of each edge within its (core, t, s) bucket
    order = np.argsort(kf, kind="stable")
    cnt_flat = np.bincount(kf, minlength=NCORES * NT * 2)
    start_flat = np.concatenate([[0], np.cumsum(cnt_flat)])[:-1]
    rank = np.empty(E, dtype=np.int64)
    rank[order] = np.arange(E) - start_flat[kf[order]]

    gslot = GPOS[t, stream] + rank                                  # [E]
    Bg = gslot >> 7
    prow = gslot & 127

    # dstslot array [cores, 128, NBTOT] bf16 (pad = -1)
    dslot_np = np.full((NCORES, P, NBTOT), -1.0, dtype=np.float32)
    dslot_np[core, prow, Bg] = slot
    dslot_np = dslot_np.astype(ml_dtypes.bfloat16)

    # parity mask [cores, 128, NBTOT] bf16 (1 -> use odd half of pair row)
    pmask_np = np.zeros((NCORES, P, NBTOT), dtype=np.float32)
    pmask_np[core, prow, Bg] = par
    pmask_np = pmask_np.astype(ml_dtypes.bfloat16)

    # idx array [cores, 128, NBTOT*8] int16, 16-wrapped per call, replicated
    icol_base = 8 * call_block_base                                 # per call
    k_e = k_of[t, stream]
    i_in_call = gslot - call_block_base[k_e] * P
    col = icol_base[k_e] + (i_in_call >> 4)
    row16 = i_in_call & 15
    idx16 = np.zeros((NCORES, 16, NBTOT * 8), dtype=np.int16)
    idx16[core, row16, col] = pairrel
    idx_np = np.tile(idx16, (1, 8, 1))

    # CSR-ish offsets for degree (device computes deg = o2 - o1 + 1)
    o1_np = np.zeros((NCORES, P, NT), dtype=np.float32)
    o2_np = np.zeros((NCORES, P, NT), dtype=np.float32)
    for c in range(NCORES):
        cnt_node = np.bincount(dloc[core == c], minlength=NPAD).astype(np.int64)
        o2v = np.cumsum(cnt_node)
        o1v = o2v - cnt_node
        o1_np[c] = o1v.reshape(NT, P).T.astype(np.float32)
        o2_np[c] = o2v.reshape(NT, P).T.astype(np.float32)

    # Xt shards [64, 12544] fp32
    xt_np = np.zeros((NCORES, IN_C, NPAD), dtype=np.float32)
    for c in range(NCORES):
        xt_np[c, :, :NSH] = X[c * NSH:(c + 1) * NSH].T

    iota_np = np.broadcast_to(np.arange(P, dtype=np.float32), (P, P)).astype(ml_dtypes.bfloat16)
    ident_np = np.eye(P, dtype=np.float32).astype(ml_dtypes.bfloat16)
    b2rep_np = np.tile(b2.reshape(1, OUT_C), (P, TG)).astype(np.float32)  # [128, TG*64]

    # per-tile matmul emission metadata (shared across cores)
    tile_blocks = []  # tile t -> list of (call k, block-in-call b)
    for tt in range(NT):
        g = tt // TG
        lst = []
        for s in range(2):
            k = 2 * g + s
            for b in range(int(NB[tt, s])):
                lst.append((k, int(seg_block_base[tt, s]) + b))
        tile_blocks.append(lst)

    meta = dict(NB=NB, NBC=NBC.astype(int), NBTOT=NBTOT, tile_blocks=tile_blocks)
    per_core = []
    for c in range(NCORES):
        per_core.append({
            "xt": np.ascontiguousarray(xt_np[c]),
            "w1": W1, "b1": b1.reshape(HID, 1), "w2": W2, "b2": b2.reshape(1, OUT_C),
            "o1": np.ascontiguousarray(o1_np[c]), "o2": np.ascontiguousarray(o2_np[c]),
            "iota": iota_np, "ident": ident_np, "b2rep": b2rep_np,
            "dslot": np.ascontiguousarray(dslot_np[c]),
            "pmask": np.ascontiguousarray(pmask_np[c]),
            "idxs": np.ascontiguousarray(idx_np[c]),
        })
    return meta, per_core


def _build_program(meta):
    SKIP_GATHER = os.environ.get("SKIP_GATHER", "0") == "1"
    SKIP_S = os.environ.get("SKIP_S", "0") == "1"
    SKIP_SCAT = os.environ.get("SKIP_SCAT", "0") == "1"
    SKIP_AG = os.environ.get("SKIP_AG", "0") == "1"
    SKIP_DENSE = os.environ.get("SKIP_DENSE", "0") == "1"
    NBC = meta["NBC"]
    NBTOT = meta["NBTOT"]
    tile_blocks = meta["tile_blocks"]

    # split each call k into SPLITS sub-calls so several desc-gens (one per
    # SWDGE queue / Q7 core pair) run concurrently, with a deep buffer
    # rotation to pipeline desc-gen against drain + DVE + PE
    SPLITS = 4
    NSETS = 16          # exactly two 8-call batches -> full double buffering
    NSETS_EARLY = 6     # sets allocated outside the dense-transient region
    OFF = []   # OFF[k][i] = block offset of sub-call i within old call k
    SZ = []
    for k in range(NCALL):
        n = int(NBC[k])
        sz = [(n + SPLITS - 1 - i) // SPLITS for i in range(SPLITS)]
        off = [0] * SPLITS
        for i in range(1, SPLITS):
            off[i] = off[i - 1] + sz[i - 1]
        OFF.append(off)
        SZ.append(sz)
    NCALL2 = SPLITS * NCALL
    NBC2 = [SZ[K // SPLITS][K % SPLITS] for K in range(NCALL2)]
    NBCMAX = max(NBC2)

    nc = bacc.Bacc("TRN2", num_devices=NCORES, num_swdge_queues=4)

    # --- DRAM tensors ---
    xt_d = nc.dram_tensor("xt", [IN_C, NPAD], FP32, kind="ExternalInput")
    w1_d = nc.dram_tensor("w1", [IN_C, HID], FP32, kind="ExternalInput")
    b1_d = nc.dram_tensor("b1", [HID, 1], FP32, kind="ExternalInput")
    w2_d = nc.dram_tensor("w2", [HID, OUT_C], FP32, kind="ExternalInput")
    o1_d = nc.dram_tensor("o1", [P, NT], FP32, kind="ExternalInput")
    o2_d = nc.dram_tensor("o2", [P, NT], FP32, kind="ExternalInput")
    iota_d = nc.dram_tensor("iota", [P, P], BF16, kind="ExternalInput")
    ident_d = nc.dram_tensor("ident", [P, P], BF16, kind="ExternalInput")
    b2rep_d = nc.dram_tensor("b2rep", [P, TG * OUT_C], FP32, kind="ExternalInput")
    dslot_d = nc.dram_tensor("dslot", [P, NBTOT], BF16, kind="ExternalInput")
    pmask_d = nc.dram_tensor("pmask", [P, NBTOT], BF16, kind="ExternalInput")
    idxs_d = nc.dram_tensor("idxs", [P, NBTOT * 8], I16, kind="ExternalInput")
    out_d = nc.dram_tensor("out", [NSH, OUT_C], FP32, kind="ExternalOutput")
    agin_d = nc.dram_tensor("agin", [NSH * OUT_C], BF16, kind="Internal")
    agout_d = nc.dram_tensor("agout", [PAIRS, P], BF16, kind="Internal",
                             addr_space="Shared")

    from contextlib import ExitStack
    stack = ExitStack()
    sem = {}
    for name in ["sin", "smm1", "srelu", "smm2", "sgn", "sagin", "scc", "sg",
                 "sS", "spe", "sactg", "sout", "swb", "scast", "sdeg", "sdinv",
                 "sbb", "sdense", "sdeg2", "sinc", "sidx", "sag1", "sag2"
                 ] + [f"sgs{i}" for i in range(16)]:
        sem[name] = stack.enter_context(nc.semaphore(name))

    # --- persistent SBUF ---
    sb = lambda name, shape, dt: stack.enter_context(nc.sbuf_tensor(name, shape, dt))
    w1_t = sb("w1t", [IN_C, HID], FP32)
    w1bf = sb("w1bf", [IN_C, HID], BF16)
    w2_t = sb("w2t", [HID, OUT_C], FP32)
    w2bf = sb("w2bf", [HID, OUT_C], BF16)
    b1_t = sb("b1t", [HID, 1], FP32)
    o1_t = sb("o1t", [P, NT], FP32)
    o2_t = sb("o2t", [P, NT], FP32)
    degm1 = sb("degm1", [P, NT], FP32)
    degp1 = sb("degp1", [P, NT], FP32)
    recip = sb("recip", [P, NT], FP32)
    dinv = sb("dinv", [P, NT], FP32)
    iota_t = sb("iotat", [P, P], BF16)
    ident_t = sb("identt", [P, P], BF16)
    b2rep_t = sb("b2rept", [P, TG * OUT_C], FP32)
    dslot_t = sb("dslott", [P, NBTOT], BF16)
    pmask_t = sb("pmaskt", [P, NBTOT], BF16)
    idx_t = sb("idxt", [P, NBTOT * 8], I16)
    g_node = sb("gnode", [P, NT, OUT_C], BF16)
    ostage = [sb(f"ost{i}", [P, TG * OUT_C], FP32) for i in range(2)]

    # PSUM: 8 banks exactly
    ps = lambda name, shape: stack.enter_context(nc.psum_tensor(name, shape, FP32))
    psum_h1 = [ps(f"ph1{i}", [P, 512]) for i in range(2)]
    psum_g = [ps(f"pg{i}", [P, 512]) for i in range(4)]
    psum_grp = [ps(f"pgrp{i}", [P, 512]) for i in range(2)]

    # the first NSETS_EARLY buffer sets are allocated below the dense
    # transients (stack allocator), so they never overlap them: their gather
    # DMAs may fire before the dense phase finishes
    gbuf = [stack.enter_context(nc.sbuf_tensor(f"gbuf{i}", [P, NBCMAX, P], BF16))
            for i in range(NSETS_EARLY)]
    sbuf_S = [stack.enter_context(nc.sbuf_tensor(f"sS{i}", [P, NBCMAX, P], BF16))
              for i in range(NSETS_EARLY)]

    # dense-phase transient SBUF (freed before gather buffers are allocated)
    xt_ctx = nc.sbuf_tensor("xtf32", [IN_C, NPAD], FP32)
    xt_f32 = xt_ctx.__enter__()
    xtbf_ctx = nc.sbuf_tensor("xtbf", [IN_C, NPAD], BF16)
    xt_bf = xtbf_ctx.__enter__()
    htT_ctx = nc.sbuf_tensor("htT", [P, NPAD], BF16)
    htT = htT_ctx.__enter__()

    n_in = 0
    blk_cm = nc.Block()
    blk = blk_cm.__enter__()

    # ---------------- phase A ----------------
    inputs_list = [
        (xt_f32, xt_d), (w1_t, w1_d), (b1_t, b1_d), (w2_t, w2_d),
        (o1_t, o1_d), (o2_t, o2_d), (iota_t, iota_d), (ident_t, ident_d),
        (b2rep_t, b2rep_d),
        (dslot_t, dslot_d), (pmask_t, pmask_d), (idx_t, idxs_d),
    ]
    n_in = len(inputs_list)

    mm1_chunks = []
    c0 = 0
    while c0 < NPAD:
        w = min(512, NPAD - c0)
        mm1_chunks.append((c0, w))
        c0 += w
    NCH = len(mm1_chunks)

    # group-completion semaphores: a prefix wait on one shared counter is
    # unsound (increments from different in-flight DMAs interleave), so each
    # dependency group gets its own sem. Order matches inputs_list.
    input_sem_names = ["sdense"] * 4 + ["sdeg2"] * 2 + ["sinc"] * 5 + ["sidx"]

    GRP_HALF = 7                             # agin groups covering first half

    def emit_sp_a(sync):
        for (tile_sb, dram), sname in zip(inputs_list, input_sem_names):
            sync.dma_start(tile_sb[:], dram[:]).then_inc(sem[sname], 16)
        # write g shard to agin (node-major rows); first-half writes signal
        # sag1, second-half sag2 (AG1 must wait on a FULL group count — a
        # prefix wait on one shared counter is unsound with in-flight DMAs)
        nag1 = nag2 = 0
        for bidx in range(NGRP):
            t0 = bidx * TG
            t1 = min(t0 + TG, NT)
            full_t1 = min(t1, 97)  # tiles 0..96 are full
            hsem = "sag1" if bidx < GRP_HALF else "sag2"
            sync.wait_ge(sem["sgn"], min(t1, NT))
            if full_t1 > t0:
                dest = agin_d[t0 * P * OUT_C: full_t1 * P * OUT_C].rearrange(
                    "(t p d) -> p t d", p=P, t=full_t1 - t0)
                sync.dma_start(dest, g_node[:, t0:full_t1, :]).then_inc(sem[hsem], 16)
                if bidx < GRP_HALF:
                    nag1 += 1
                else:
                    nag2 += 1
            if t1 > 97:
                dest = agin_d[97 * P * OUT_C: NSH * OUT_C].rearrange(
                    "(p d) -> p d", p=LAST_ROWS)
                sync.dma_start(dest, g_node[:LAST_ROWS, 97, :]).then_inc(sem[hsem], 16)
                nag2 += 1
        emit_sp_a.nag1 = nag1
        emit_sp_a.nag2 = nag2

    blk.sync(emit_sp_a)
    NAG1 = emit_sp_a.nag1
    NAG2 = emit_sp_a.nag2

    def emit_dve_a(vector):
        vector.wait_ge(sem["sdense"], 16 * 4)   # xt, w1, b1, w2
        vector.tensor_copy(w1bf[:], w1_t[:])
        vector.tensor_copy(w2bf[:], w2_t[:])
        vector.nop().then_inc(sem["swb"], 1)
        vector.tensor_copy(xt_bf[:], xt_f32[:]).then_inc(sem["scast"], 1)
        vector.wait_ge(sem["sdeg2"], 16 * 2)    # o1, o2
        vector.tensor_tensor(out=degm1[:], in0=o2_t[:], in1=o1_t[:],
                             op=mybir.AluOpType.subtract)
        vector.tensor_scalar_add(degp1[:], degm1[:], 1.0)
        vector.reciprocal(recip[:], degp1[:]).then_inc(sem["sdeg"], 1)
        for j in range(NT):
            vector.wait_ge(sem["smm2"], j + 1)
            vector.tensor_scalar_mul(
                g_node[:, j, :], psum_g[j % 4][:, :OUT_C], dinv[:, j:j + 1]
            ).then_inc(sem["sgn"], 1)

    blk.vector(emit_dve_a)

    def emit_act_a(scalar):
        scalar.wait_ge(sem["sdense"], 16 * 4)
        scalar.wait_ge(sem["sdeg"], 1)
        scalar.activation(dinv[:], recip[:], mybir.ActivationFunctionType.Sqrt
                          ).then_inc(sem["sdinv"], 1)
        for ci, (cst, w) in enumerate(mm1_chunks):
            scalar.wait_ge(sem["smm1"], ci + 1)
            scalar.activation(htT[:, cst:cst + w], psum_h1[ci % 2][:, :w],
                              mybir.ActivationFunctionType.Relu,
                              bias=b1_t[:]).then_inc(sem["srelu"], 1)

    blk.scalar(emit_act_a)

    def emit_pe_a(tensor):
        tensor.wait_ge(sem["sdense"], 16 * 4)
        tensor.wait_ge(sem["swb"], 1)
        tensor.wait_ge(sem["scast"], 1)
        for ci, (cst, w) in enumerate(mm1_chunks):
            if ci >= 2:
                tensor.wait_ge(sem["srelu"], ci - 1)
            if SKIP_DENSE:
                tensor.nop().then_inc(sem["smm1"], 1)
                continue
            tensor.matmul(out=psum_h1[ci % 2][:, :w], lhsT=w1bf[:],
                          rhs=xt_bf[:, cst:cst + w], start=True,
                          stop=True).then_inc(sem["smm1"], 1)
        last_relu_wait = 0
        for j in range(NT):
            if j >= 4:
                tensor.wait_ge(sem["sgn"], j - 3)
            need = (j * P + P - 1) // 512 + 1
            if need > last_relu_wait:
                tensor.wait_ge(sem["srelu"], need)
                last_relu_wait = need
            tensor.matmul(out=psum_g[j % 4][:, :OUT_C],
                          lhsT=htT[:, j * P:(j + 1) * P], rhs=w2bf[:],
                          start=True, stop=True).then_inc(sem["smm2"], 1)

    blk.tensor(emit_pe_a)

    HALF_ELEMS = (NSH // 2) * OUT_C          # 6250*64 elems per half-shard
    GRP_HALF = 7                             # agin groups covering first half

    def emit_gp_a(gpsimd):
        gpsimd.load_library(mlp_library)
        if SKIP_AG:
            gpsimd.wait_ge(sem["sag1"], 16 * NAG1)
            gpsimd.wait_ge(sem["sag2"], 16 * NAG2)
            gpsimd.nop().then_inc(sem["scc"], 2)
        else:
            gpsimd.wait_ge(sem["sag1"], 16 * NAG1)
            gpsimd.collective_compute(
                "AllGather", mybir.AluOpType.bypass,
                replica_groups=[list(range(NCORES))],
                ins=[agin_d[:HALF_ELEMS].opt()],
                outs=[agout_d[:CHUNK, :].opt()],
            ).then_inc(sem["scc"], 1)
            gpsimd.wait_ge(sem["sag2"], 16 * NAG2)
            gpsimd.collective_compute(
                "AllGather", mybir.AluOpType.bypass,
                replica_groups=[list(range(NCORES))],
                ins=[agin_d[HALF_ELEMS:].opt()],
                outs=[agout_d[CHUNK:, :].opt()],
            ).then_inc(sem["scc"], 1)

    blk.gpsimd(emit_gp_a)

    # free dense transients, allocate the remaining sets in their place
    htT_ctx.__exit__(None, None, None)
    xtbf_ctx.__exit__(None, None, None)
    xt_ctx.__exit__(None, None, None)

    gbuf += [stack.enter_context(nc.sbuf_tensor(f"gbuf{i}", [P, NBCMAX, P], BF16))
             for i in range(NSETS_EARLY, NSETS)]
    sbuf_S += [stack.enter_context(nc.sbuf_tensor(f"sS{i}", [P, NBCMAX, P], BF16))
               for i in range(NSETS_EARLY, NSETS)]

    # ---------------- phase B ----------------
    call_block_base = np.concatenate([[0], np.cumsum(NBC)])[:-1]

    def emit_gp_b(gpsimd):
        gpsimd.wait_ge(sem["sidx"], 16)         # idx table loaded
        for K in range(NCALL2):
            if K == NSETS_EARLY:
                # sets >= NSETS_EARLY reuse the SBUF freed from xt/htT: their
                # gather DMAs may not land until the dense phase has fully
                # consumed those transients
                gpsimd.wait_ge(sem["sgn"], NT)
            k, h = K // SPLITS, K % SPLITS
            c = k % 2
            nb = int(NBC2[K])
            if K < 2 * SPLITS:
                gpsimd.wait_ge(sem["scc"], c + 1)
            if nb == 0:
                gpsimd.nop().then_inc(sem[f"sgs{K % NSETS}"], 16)
                continue
            if K >= NSETS:
                gpsimd.wait_ge(sem["spe"], (K - NSETS) // (2 * SPLITS) + 1)
            b0 = int(call_block_base[k]) + OFF[k][h]
            colb = 8 * b0
            if SKIP_GATHER:
                gpsimd.nop().then_inc(sem[f"sgs{K % NSETS}"], 16)
            else:
                gpsimd.dma_gather(
                    gbuf[K % NSETS][:, :nb, :],
                    agout_d[c * CHUNK:(c + 1) * CHUNK, :],
                    idx_t[:, colb:colb + nb * 8],
                    nb * P, nb * P, P,
                    single_packet=False,
                    queue_num=K % 4,
                ).then_inc(sem[f"sgs{K % NSETS}"], 16)
        for s in range(NSETS):
            gpsimd.wait_ge(sem[f"sgs{s}"],
                           16 * ((NCALL2 - 1 - s) // NSETS + 1))

    def emit_dve_b(vector):
        vector.wait_ge(sem["sinc"], 16 * 5)     # iota/ident/b2rep/dslot/pmask

        def s_build(K):
            k, h = K // SPLITS, K % SPLITS
            nb = int(NBC2[K])
            if nb == 0:
                vector.nop().then_inc(sem["sS"], 1)
                return
            if K >= NSETS:
                vector.wait_ge(sem["spe"], (K - NSETS) // (2 * SPLITS) + 1)
            B0 = int(call_block_base[k]) + OFF[k][h]
            if SKIP_S:
                vector.nop().then_inc(sem["sS"], 1)
                return
            vector.tensor_tensor(
                out=sbuf_S[K % NSETS][:, :nb, :],
                in0=dslot_t[:, B0:B0 + nb, None].to_broadcast([P, nb, P]),
                in1=iota_t[:, None, :].to_broadcast([P, nb, P]),
                op=mybir.AluOpType.is_equal,
            )
            # in-place parity select: lo += (hi - lo) * pmask (hi half used
            # as scratch; result lands in the lo half). Per-SET DMA sem: sets
            # are strictly serialized by the spe recycle wait, so at most one
            # gather per set is in flight and the full-count wait is sound.
            g = gbuf[K % NSETS]
            vector.wait_ge(sem[f"sgs{K % NSETS}"], 16 * (K // NSETS + 1))
            vector.tensor_tensor(
                out=g[:, :nb, OUT_C:],
                in0=g[:, :nb, OUT_C:],
                in1=g[:, :nb, :OUT_C],
                op=mybir.AluOpType.subtract,
            )
            vector.tensor_tensor(
                out=g[:, :nb, OUT_C:],
                in0=g[:, :nb, OUT_C:],
                in1=pmask_t[:, B0:B0 + nb, None].to_broadcast([P, nb, OUT_C]),
                op=mybir.AluOpType.mult,
            )
            vector.tensor_tensor(
                out=g[:, :nb, :OUT_C],
                in0=g[:, :nb, :OUT_C],
                in1=g[:, :nb, OUT_C:],
                op=mybir.AluOpType.add,
            ).then_inc(sem["sS"], 1)

        def bias_add(g):
            vector.wait_ge(sem["sactg"], g + 1)
            vector.tensor_tensor(
                out=ostage[g % 2][:], in0=ostage[g % 2][:], in1=b2rep_t[:],
                op=mybir.AluOpType.add).then_inc(sem["sbb"], 1)

        for g in range(NGRP):
            for j in range(2 * SPLITS):
                s_build(2 * SPLITS * g + j)
            if g >= 2:
                bias_add(g - 2)
        bias_add(NGRP - 2)
        bias_add(NGRP - 1)

    def emit_pe_b(tensor):
        tensor.wait_ge(sem["sinc"], 16 * 5)     # ident loaded
        tensor.wait_ge(sem["sgn"], NT)
        for g in range(NGRP):
            if g >= 2:
                tensor.wait_ge(sem["sactg"], g - 1)
            tensor.wait_ge(sem["sS"], 2 * SPLITS * (g + 1))
            last = None
            for tl in range(TG):
                t = g * TG + tl
                pcol = psum_grp[g % 2][:, tl * OUT_C:(tl + 1) * OUT_C]
                first = True
                if not SKIP_SCAT:
                    for (k, b) in tile_blocks[t]:
                        h = 0
                        while h + 1 < SPLITS and b >= OFF[k][h + 1]:
                            h += 1
                        K = SPLITS * k + h
                        b2 = b - OFF[k][h]
                        last = tensor.matmul(
                            out=pcol,
                            lhsT=sbuf_S[K % NSETS][:, b2, :],
                            rhs=gbuf[K % NSETS][:, b2, :OUT_C],
                            start=first, stop=False)
                        first = False
                last = tensor.matmul(out=pcol, lhsT=ident_t[:],
                                     rhs=g_node[:, t, :], start=first, stop=True)
            last.then_inc(sem["spe"], 1)

    def emit_act_b(scalar):
        for g in range(NGRP):
            scalar.wait_ge(sem["spe"], g + 1)
            if g >= 2:
                scalar.wait_ge(sem["sout"], 16 * (g - 1))
            last = None
            for tl in range(TG):
                t = g * TG + tl
                rows = P if t < 97 else LAST_ROWS
                last = scalar.activation(
                    ostage[g % 2][:rows, tl * OUT_C:(tl + 1) * OUT_C],
                    psum_grp[g % 2][:rows, tl * OUT_C:(tl + 1) * OUT_C],
                    mybir.ActivationFunctionType.Copy,
                    scale=dinv[:rows, t:t + 1])
            last.then_inc(sem["sactg"], 1)

    def emit_sp_b(sync):
        nout = 0
        for g in range(NGRP):
            sync.wait_ge(sem["sbb"], g + 1)
            t0 = g * TG
            t1 = min(t0 + TG, NT)
            full_t1 = min(t1, 97)
            if full_t1 > t0:
                dest = out_d[t0 * P:full_t1 * P, :].rearrange(
                    "(t p) d -> p t d", p=P)
                sync.dma_start(dest, ostage[g % 2][:, : (full_t1 - t0) * OUT_C]
                               .rearrange("p (t d) -> p t d", d=OUT_C)
                               ).then_inc(sem["sout"], 16)
                nout += 1
            if t1 > 97:
                dest = out_d[97 * P:NSH, :]
                sync.dma_start(
                    dest,
                    ostage[g % 2][:LAST_ROWS, (97 - t0) * OUT_C:(98 - t0) * OUT_C],
                ).then_inc(sem["sout"], 16)
                nout += 1
        sync.wait_ge(sem["sout"], 16 * nout)

    blk.gpsimd(emit_gp_b)
    blk.vector(emit_dve_b)
    blk.tensor(emit_pe_b)
    blk.scalar(emit_act_b)
    blk.sync(emit_sp_b)

    blk_cm.__exit__(None, None, None)
    stack.close()
    nc.finalize()
    return nc


def kernel(node_features, edge_index, W1, b1, W2, b2):
    meta, per_core = _host_prep(node_features, edge_index, W1, b1, W2, b2)
    nc = _build_program(meta)
    trace = os.environ.get("GCN_TRACE", "0") == "1"
    res = run_bass_kernel_spmd(nc, per_core, core_ids=list(range(NCORES)),
                               trace=trace)
    LAST_EXEC_NS[0] = res.exec_time_ns
    LAST_TRACE[0] = res.instructions_and_trace
    out = np.concatenate([np.asarray(res.results[c]["out"]) for c in range(NCORES)],
                         axis=0)
    return out.astype(np.float32)

